# revision 43
# baseline (speedup 1.0000x reference)
"""Multi-head cross attention on 8 Trainium2 NeuronCores.

Sharding: core c = b*4 + g handles batch b (of 2) and head-group g (4 heads
of the 16).  Each core projects Q/K/V for its 4 heads, runs attention, and
computes a partial output projection with its 256 rows of Wo; the host sums
the 4 partials per batch (plus bo and the bv@Wo term, exact because softmax
rows sum to 1).

Schedule (v3): the ACT engine's exp over 16.8M score elements (~147us at
1 elem/cyc/lane) is the per-core bottleneck; the whole schedule exists to
start that exp stream early and never let it gap:
  - inputs stream as paired-d tiles (xkv on the fast sync DMA queue, xq on
    gpsimd); K projection runs 2 rounds over 4 PSUM banks as tiles land,
    then only the qq0 quarter of the Q projection -> first exp at ~24us
  - attention uses a one-block software pipeline: block k emits scores+exps
    for block k and the AV matmuls of block k-1 (p_t tiles carry over), so
    exp never waits on the AV/normalize chain at block boundaries
  - the PE's per-j slack under ACT (~0.5us) is packed with fillers:
    V' projection in block 0, Q projection qc1-3 in blocks 1-2, the output
    projection of finished qq groups in blocks 3/5, block 7's own AVs in
    block 7; outproj(qq2)+(qq3) drain at the end
  - ACT does exp ONLY (table preloaded via a dummy exp at t=0); KT/QT bias
    adds run on DVE, V'-ones fixups on GpSimd memset, softmax normalize is
    reciprocal_approx_fast + partition-broadcast + DVE multiply
PSUM: st 2x[128,1024] (4 banks) + o_ps pair (2) + aux (2) shared serially
by {pv, woven pq, po, block-7 o_ps pair}.
Dataflow is fully transposed (see build_in_maps): QT/KT = W.T @ xT,
V' = xkvT.T @ Wv' with a ones column per head so the AV matmul also emits
softmax row-sums.  Matmuls in bf16, fp32 PSUM accumulation.
"""

import sys

sys.path.insert(0, "/opt/trn_rl_repo")

import ml_dtypes
import numpy as np

BF16NP = ml_dtypes.bfloat16

B, SQ, SKV, D, H = 2, 2048, 2048, 1024, 16
DH = D // H          # 64
N_CORES = 8
G = 4                # head groups
HPG = H // G         # heads per group = 4
GC = HPG * DH        # group width = 256

_nc_cache = None


def _build_nc():
    import concourse.mybir as mybir
    import concourse.tile as tile
    from concourse import bacc

    F32 = mybir.dt.float32
    BF16 = mybir.dt.bfloat16
    AF = mybir.ActivationFunctionType
    MUL = mybir.AluOpType.mult

    nc = bacc.Bacc("TRN2", target_bir_lowering=False, debug=False,
                   num_devices=N_CORES)

    xqT_d = nc.dram_tensor("xqT", [D, SQ], BF16, kind="ExternalInput").ap()
    xkvT_d = nc.dram_tensor("xkvT", [D, SKV], BF16, kind="ExternalInput").ap()
    wk_d = nc.dram_tensor("wk", [D, GC], BF16, kind="ExternalInput").ap()
    # packed [Wq | Wv'] (Wv' has a zero column after each head's 64, the
    # slot for the ones column); packing keeps the total input-DMA count
    # at 16 so the runtime's shared DMA-semaphore pool never chains one
    # queue's loads behind another's
    WQV = GC + HPG * 65          # 516
    wqvp_d = nc.dram_tensor("wqvp", [D, WQV], BF16, kind="ExternalInput").ap()
    wo_d = nc.dram_tensor("wo", [GC, D], BF16, kind="ExternalInput").ap()
    b2_d = nc.dram_tensor("b2", [128, 4], F32, kind="ExternalInput").ap()
    # legacy input, unused on-device but still part of the host contract
    nc.dram_tensor("ones64", [1, 128], mybir.dt.float32r, kind="ExternalInput")
    out_d = nc.dram_tensor("out_p", [SQ, D], F32, kind="ExternalOutput").ap()
    _DBG = bool(globals().get("_DEBUG_DUMPS"))
    if _DBG:
        dbg_kt = nc.dram_tensor("dbg_kt", [128, 2 * SKV], BF16,
                                kind="ExternalOutput").ap()
        dbg_qt = nc.dram_tensor("dbg_qt", [128, 2 * SQ], BF16,
                                kind="ExternalOutput").ap()
        dbg_vp = nc.dram_tensor("dbg_vp", [128, (SKV // 128) * HPG * 65 + 63],
                                BF16, kind="ExternalOutput").ap()
        dbg_oA = nc.dram_tensor("dbg_oA", [128, 2 * 1024], BF16,
                                kind="ExternalOutput").ap()
        dbg_oB = nc.dram_tensor("dbg_oB", [128, 2 * 1024], BF16,
                                kind="ExternalOutput").ap()

    ND = D // 128        # 8 d-tiles (contraction over D)
    NP = ND // 2         # 4 paired-d input tiles
    NJ = SKV // 128      # 16 kv tiles
    VW = HPG * 65        # 260, V' row width
    scale = 1.0 / float(np.sqrt(DH))

    with tile.TileContext(nc) as tc:
        with (
            tc.tile_pool(name="persist", bufs=1) as pp,
            tc.tile_pool(name="attn", bufs=1) as at,
        ):
            # ---- persistent tiles -------------------------------------
            qt_sb = pp.tile([128, 2 * SQ], BF16, tag="qt_sb")
            kt_sb = pp.tile([128, 2 * SKV], BF16, tag="kt_sb")
            vp_sb = pp.tile([128, NJ * VW + 63], BF16, tag="vp_sb")
            # normalized O, per qq pair/quarter (split tiles so late norm
            # writes don't false-WAR earlier outproj reads)
            o_sbA = pp.tile([128, 2 * 1024], BF16, tag="o_sbA")
            o_sbB2 = pp.tile([128, 2 * 512], BF16, tag="o_sbB2")
            o_sbB3 = pp.tile([128, 2 * 512], BF16, tag="o_sbB3")
            b2_sb = pp.tile([128, 4], F32, tag="b2_sb")
            wk_sb = pp.tile([128, ND * GC], BF16, tag="wk_sb")
            wqvp_sb = pp.tile([128, ND * WQV], BF16, tag="wqvp_sb")
            wo_sb = pp.tile([128, 2 * D], BF16, tag="wo_sb")
            warm = pp.tile([1, 32], F32, tag="warm")
            warm2 = pp.tile([128, 640], BF16, tag="warm2")

            nc.gpsimd.memset(warm[:], 0.0)
            nc.gpsimd.memset(warm2[:], 0.0)
            # zero the 63-col tail pad of V' (AV lhsT windows over-read it)
            nc.gpsimd.memset(vp_sb[:, NJ * VW:NJ * VW + 63], 0.0)

            # ---- DMA issue (16 total; order = priority) ----------------
            # One queue carries the big activation stream IN NEED ORDER so
            # xq never steals HBM bandwidth from xkv (fair-share across
            # queues would gate the K projection on the whole stream):
            # sync: wk, xkv x5, xq x5.  gpsimd: b2, wqvp, wo.
            nc.sync.dma_start(
                out=wk_sb[:].rearrange("p (t n) -> p t n", t=ND),
                in_=wk_d.rearrange("(t p) n -> p t n", p=128))
            xkv = []
            for d in range(4):
                t = pp.tile([128, SKV], BF16, tag=f"xkv{d}", name=f"xkv{d}")
                nc.sync.dma_start(out=t[:],
                                  in_=xkvT_d[d * 128:(d + 1) * 128, :])
                xkv.append(t)
            xkvq = pp.tile([128, 4 * SKV], BF16, tag="xkvq", name="xkvq")
            nc.sync.dma_start(
                out=xkvq[:].rearrange("p (t n) -> p t n", t=4),
                in_=xkvT_d[512:1024, :].rearrange("(t p) n -> p t n", p=128))
            xq = []
            for d in range(4):
                t = pp.tile([128, SQ], BF16, tag=f"xq{d}", name=f"xq{d}")
                nc.sync.dma_start(out=t[:],
                                  in_=xqT_d[d * 128:(d + 1) * 128, :])
                xq.append(t)
            xqq = pp.tile([128, 4 * SQ], BF16, tag="xqq", name="xqq")
            nc.sync.dma_start(
                out=xqq[:].rearrange("p (t n) -> p t n", t=4),
                in_=xqT_d[512:1024, :].rearrange("(t p) n -> p t n", p=128))
            # gpsimd queue: biases, packed wq|wvp, wo
            nc.gpsimd.dma_start(out=b2_sb[:], in_=b2_d[:])
            nc.gpsimd.dma_start(
                out=wqvp_sb[:].rearrange("p (t n) -> p t n", t=ND),
                in_=wqvp_d.rearrange("(t p) n -> p t n", p=128))
            nc.gpsimd.dma_start(
                out=wo_sb[:].rearrange("p (t n) -> p t n", t=2),
                in_=wo_d.rearrange("(t p) n -> p t n", p=128))
            # preload the exp spline tables while DMAs stream
            nc.scalar.activation(warm[:], warm[:], AF.Exp)

            def xkv_ap(d, lo, hi):
                if d < 4:
                    return xkv[d][:, lo:hi]
                return xkvq[:, (d - 4) * SKV + lo:(d - 4) * SKV + hi]

            def xq_ap(d, lo, hi):
                if d < 4:
                    return xq[d][:, lo:hi]
                return xqq[:, (d - 4) * SQ + lo:(d - 4) * SQ + hi]

            # ---- K projection (2 rounds over 4 PSUM banks) ------------
            with tc.tile_pool(name="psA", bufs=1, space="PSUM") as psA:
                # warm-up dummies: ~8us of back-to-back zero matmuls keep
                # the PE's HAM clock gate at 2.4 GHz through the DMA wait,
                # so the arrival-chained projections run at full clock
                wup = psA.tile([128, 512], F32, tag="pk", bufs=8,
                               name="wup")
                for _ in range(36):
                    nc.tensor.matmul(wup[:], warm2[:, 0:128],
                                     warm2[:, 128:640],
                                     start=True, stop=True)
                # single pass, 8 accumulators (psA owns all 8 banks and
                # closes before psC opens), consuming xkv tiles on arrival
                pk = {}
                for p in range(2):
                    for qc in range(4):
                        pk[p, qc] = psA.tile([128, 512], F32, tag="pk",
                                             bufs=8, name=f"pk{p}{qc}")
                for d in range(ND):
                    for p in range(2):
                        for qc in range(4):
                            nc.tensor.matmul(
                                pk[p, qc][:],
                                wk_sb[:, d * GC + p * 128:d * GC + (p + 1) * 128],
                                xkv_ap(d, qc * 512, (qc + 1) * 512),
                                start=(d == 0), stop=(d == ND - 1),
                            )
                for p in range(2):
                    for qc in range(4):
                        nc.vector.tensor_scalar_add(
                            kt_sb[:, p * SKV + qc * 512:p * SKV + (qc + 1) * 512],
                            pk[p, qc][:], b2_sb[:, 2 + p:3 + p])

            # ---- attention (one-block AV-shift pipeline) --------------
            with tc.tile_pool(name="psC", bufs=1, space="PSUM") as psC:
                blocks = [(qq, t) for qq in range(4) for t in range(2)]
                pt_store = {}
                o_pair = {}
                pending_norm = []

                def o_half_col(qq, t):
                    if qq < 2:
                        return o_sbA, t * 1024 + qq * 512
                    return (o_sbB2 if qq == 2 else o_sbB3), t * 512

                def emit_qproj_qc_mm(qc, d, pq):
                    # pq: {p: AP of a [128,512] fp32 PSUM accumulator}
                    for p in range(2):
                        nc.tensor.matmul(
                            pq[p],
                            wqvp_sb[:, d * WQV + p * 128:d * WQV + (p + 1) * 128],
                            xq_ap(d, qc * 512, (qc + 1) * 512),
                            start=(d == 0), stop=(d == ND - 1),
                        )

                def emit_qproj_qc_add(qc, pq):
                    for p in range(2):
                        nc.vector.tensor_scalar_add(
                            qt_sb[:, p * SQ + qc * 512:p * SQ + (qc + 1) * 512],
                            pq[p], b2_sb[:, p:p + 1])

                def flush_norm():
                    while pending_norm:
                        pending_norm.pop(0)()

                def emit_score_exp(k, j):
                    qq, t = blocks[k]
                    st = psC.tile([128, 1024], F32, tag="st2", bufs=2,
                                  name=f"st{k}{j}")
                    # two heads on disjoint PE row groups, concurrent
                    for hp in range(2):
                        nc.tensor.matmul(
                            st[:, hp * 512:(hp + 1) * 512],
                            kt_sb[hp * 64:(hp + 1) * 64,
                                  t * SKV + j * 128:t * SKV + (j + 1) * 128],
                            qt_sb[hp * 64:(hp + 1) * 64,
                                  t * SQ + qq * 512:t * SQ + (qq + 1) * 512],
                            start=True, stop=True,
                        )
                    p_t = at.tile([128, 1024], BF16, tag="pt",
                                  bufs=20, name=f"pt{k}{j}")
                    nc.scalar.activation(p_t[:], st[:], AF.Exp, scale=scale)
                    pt_store[k, j] = p_t

                def emit_av(k, j):
                    qq, t = blocks[k]
                    if k not in o_pair:
                        tag = "aux" if k == 7 else "o_ps"
                        o_pair[k] = {
                            hp: psC.tile([128, 512], F32, tag=tag, bufs=2,
                                         name=f"ops{k}{hp}")
                            for hp in range(2)}
                    p_t = pt_store.pop((k, j))
                    for hp in range(2):
                        h = 2 * t + hp
                        nc.tensor.matmul(
                            o_pair[k][hp][:],
                            vp_sb[:, j * VW + h * 65:j * VW + h * 65 + 128],
                            p_t[:, hp * 512:(hp + 1) * 512],
                            start=(j == 0), stop=(j == NJ - 1),
                        )

                def emit_norm(k, drain=False):
                    # AV(k) fully emitted: stage O'+rowsum, defer normalize
                    qq, t = blocks[k]
                    for hp in range(2):
                        ot = at.tile([64, 512], F32, tag="ot", bufs=4,
                                     name=f"ot{k}{hp}")
                        nc.vector.tensor_copy(ot[:], o_pair[k][hp][0:64, :])
                        rs = at.tile([1, 512], F32, tag="rs", bufs=4,
                                     name=f"rs{k}{hp}")
                        if drain:
                            # ACT is idle once the exp stream is done
                            nc.scalar.activation(rs[:],
                                                 o_pair[k][hp][64:65, :],
                                                 AF.Copy)
                        else:
                            nc.vector.tensor_copy(rs[:],
                                                  o_pair[k][hp][64:65, :])

                        def norm(qq=qq, t=t, hp=hp, ot=ot, rs=rs):
                            rcp = at.tile([1, 512], F32, tag="rcp", bufs=4,
                                          name=f"rcp{qq}{t}{hp}")
                            nc.vector.reciprocal_approx_fast(
                                out=rcp[:], in_=rs[:])
                            bcs = at.tile([64, 512], F32, tag="bcs", bufs=4,
                                          name=f"bcs{qq}{t}{hp}")
                            nc.gpsimd.partition_broadcast(
                                bcs[:], rcp[:], channels=64)
                            o_half, col = o_half_col(qq, t)
                            nc.vector.tensor_tensor(
                                out=o_half[hp * 64:(hp + 1) * 64,
                                           col:col + 512],
                                in0=ot[:], in1=bcs[:], op=MUL)

                        pending_norm.append(norm)

                def emit_vproj(j):
                    pv = psC.tile([128, 512], F32, tag="aux", bufs=2,
                                  name=f"pv{j}")
                    for d in range(ND):
                        nc.tensor.matmul(
                            pv[:, 0:VW],
                            xkv_ap(d, j * 128, (j + 1) * 128),
                            wqvp_sb[:, d * WQV + GC:d * WQV + GC + VW],
                            start=(d == 0), stop=(d == ND - 1),
                        )
                    nc.vector.tensor_copy(vp_sb[:, j * VW:(j + 1) * VW],
                                          pv[:, 0:VW])
                    nc.gpsimd.memset(
                        vp_sb[:, j * VW + 64:(j + 1) * VW:65], 1.0)

                ob_group = {}

                def emit_outproj_tile(s, n2, drain=False):
                    po = psC.tile([128, 512], F32, tag="aux", bufs=2,
                                  name=f"po{s}{n2}")
                    qq = s // 4
                    for tt in range(2):
                        o_half, col = o_half_col(qq, tt)
                        lhs_lo = col + (s % 4) * 128
                        nc.tensor.matmul(
                            po[:],
                            o_half[:, lhs_lo:lhs_lo + 128],
                            wo_sb[:, tt * D + n2 * 512:tt * D + n2 * 512 + 512],
                            start=(tt == 0), stop=(tt == 1),
                        )
                    g = s // 4
                    if (g, n2) not in ob_group:
                        ob_group[g, n2] = at.tile([128, 4 * 512], F32,
                                                  tag="ob4", bufs=2,
                                                  name=f"ob4_{g}{n2}")
                    dst = ob_group[g, n2][:, (s % 4) * 512:(s % 4 + 1) * 512]
                    if drain:
                        # ACT is idle after the last exp; keep DVE free for
                        # the normalize chains
                        nc.scalar.activation(dst, po[:], AF.Copy)
                    else:
                        nc.vector.tensor_copy(dst, po[:])

                def emit_out_dma(g, n2):
                    # one descriptor for 4 q-tiles x 512 cols
                    ob4 = ob_group.pop((g, n2))
                    nc.sync.dma_start(
                        out=out_d[g * 512:(g + 1) * 512,
                                  n2 * 512:(n2 + 1) * 512].rearrange(
                                      "(t p) n -> p t n", p=128),
                        in_=ob4[:].rearrange("p (t n) -> p t n", t=4))

                # ---- pre-attention: V' tiles 0-3 hide the xq DMA tail,
                # then the qq0 quarter of the Q projection (st2-tag banks,
                # since psA is closed) ----------------------------------
                for j in range(4):
                    emit_vproj(j)
                pq0_t = psC.tile([128, 1024], F32, tag="st2", bufs=2,
                                 name="pq0")
                pq0 = {p: pq0_t[:, p * 512:(p + 1) * 512] for p in range(2)}
                for d in range(ND):
                    emit_qproj_qc_mm(0, d, pq0)
                emit_qproj_qc_add(0, pq0)

                # filler plans: {block: {j: [thunks]}}
                fillers = {k: {} for k in range(8)}
                # block 0: V' tiles 4-15 at js 0-11, Qproj qc1 at js 12-15
                for j in range(4, NJ):
                    fillers[0][j - 4] = [lambda j=j: emit_vproj(j)]
                pq_w = {}

                def qproj_slot(qc, dd):
                    if dd == 0:
                        pq_w[qc] = {p: psC.tile([128, 512], F32, tag="aux",
                                                bufs=2, name=f"pqw{qc}{p}")[:]
                                    for p in range(2)}
                    emit_qproj_qc_mm(qc, 2 * dd, pq_w[qc])
                    emit_qproj_qc_mm(qc, 2 * dd + 1, pq_w[qc])
                    if dd == 3:
                        emit_qproj_qc_add(qc, pq_w[qc])
                        del pq_w[qc]

                for dd in range(4):
                    fillers[0].setdefault(12 + dd, []).append(
                        lambda dd=dd: qproj_slot(1, dd))
                # blocks 1-2: Qproj qc2/qc3, one d-pair every other j
                for blk, qc in ((1, 2), (2, 3)):
                    for dd in range(4):
                        fillers[blk].setdefault(2 * dd, []).append(
                            lambda qc=qc, dd=dd: qproj_slot(qc, dd))
                # blocks 3/5: output projection of qq0/qq1 at js 2..11
                for blk, qq in ((3, 0), (5, 1)):
                    for n2 in range(2):
                        for i in range(4):
                            fillers[blk].setdefault(2 + n2 * 5 + i, []).append(
                                lambda s=qq * 4 + i, n2=n2:
                                emit_outproj_tile(s, n2))
                        fillers[blk].setdefault(6 + n2 * 5, []).append(
                            lambda g=qq, n2=n2: emit_out_dma(g, n2))
                # block 7: its own AVs ride along (aux o_ps pair)
                for j in range(1, NJ):
                    fillers[7].setdefault(j, []).append(
                        lambda j=j - 1: emit_av(7, j))

                for k in range(8):
                    for j in range(NJ):
                        emit_score_exp(k, j)
                        # AVs of the previous block, one slot behind so the
                        # o_ps pair WAR-wait hides under an exp period
                        if k >= 1 and j >= 1:
                            emit_av(k - 1, j - 1)
                        for thunk in fillers[k].get(j, ()):
                            thunk()
                        if j == 1:
                            flush_norm()
                    if k >= 1:
                        emit_av(k - 1, NJ - 1)
                        if k != 7:
                            emit_norm(k - 1)

                # ---- drain ------------------------------------------------
                # norm chains (DVE/GpSimd/ACT) for b6/b7 run concurrently
                # with the qq2 output projection (PE); drain po tiles come
                # from the now-free st2 banks so the PE never goes idle
                emit_av(7, NJ - 1)
                emit_norm(7, drain=True)
                emit_norm(6, drain=True)
                flush_norm()
                for g in (2, 3):
                    for i in range(4):
                        s = g * 4 + i
                        po = psC.tile([128, 1024], F32, tag="st2", bufs=2,
                                      name=f"poD{s}")
                        for n2 in range(2):
                            for tt in range(2):
                                o_half, col = o_half_col(g, tt)
                                lhs_lo = col + i * 128
                                nc.tensor.matmul(
                                    po[:, n2 * 512:(n2 + 1) * 512],
                                    o_half[:, lhs_lo:lhs_lo + 128],
                                    wo_sb[:, tt * D + n2 * 512:
                                          tt * D + n2 * 512 + 512],
                                    start=(tt == 0), stop=(tt == 1),
                                )
                        for n2 in range(2):
                            if (g, n2) not in ob_group:
                                ob_group[g, n2] = at.tile(
                                    [128, 4 * 512], F32, tag="ob4", bufs=2,
                                    name=f"ob4_{g}{n2}")
                            dst = ob_group[g, n2][:, i * 512:(i + 1) * 512]
                            src = po[:, n2 * 512:(n2 + 1) * 512]
                            if g == 2 or n2 == 0:
                                nc.scalar.activation(dst, src, AF.Copy)
                            else:
                                nc.vector.tensor_copy(dst, src)
                    for n2 in range(2):
                        emit_out_dma(g, n2)

                if _DBG:
                    nc.sync.dma_start(out=dbg_kt[:], in_=kt_sb[:])
                    nc.sync.dma_start(out=dbg_qt[:], in_=qt_sb[:])
                    nc.sync.dma_start(out=dbg_vp[:], in_=vp_sb[:])
                    nc.sync.dma_start(out=dbg_oA[:], in_=o_sbA[:])
                    for t in range(2):
                        nc.sync.dma_start(
                            out=dbg_oB[:, t * 1024:t * 1024 + 512],
                            in_=o_sbB2[:, t * 512:(t + 1) * 512])
                        nc.sync.dma_start(
                            out=dbg_oB[:, t * 1024 + 512:(t + 1) * 1024],
                            in_=o_sbB3[:, t * 512:(t + 1) * 512])

    nc.compile()
    return nc


def build_in_maps(inputs):
    query_input = np.asarray(inputs["query_input"], dtype=np.float32)
    kv_input = np.asarray(inputs["kv_input"], dtype=np.float32)
    Wq = np.asarray(inputs["Wq"], dtype=np.float32)
    bq = np.asarray(inputs["bq"], dtype=np.float32)
    Wkv = np.asarray(inputs["Wkv"], dtype=np.float32)
    bkv = np.asarray(inputs["bkv"], dtype=np.float32)
    Wo = np.asarray(inputs["Wo"], dtype=np.float32)

    Wk = Wkv[:, :D]
    Wv = Wkv[:, D:]
    bk = bkv[:D]
    ones64 = np.ones((1, 128), np.float32)

    xT = [np.ascontiguousarray(query_input[b].T).astype(BF16NP) for b in range(B)]
    kvT = [np.ascontiguousarray(kv_input[b].T).astype(BF16NP) for b in range(B)]

    in_maps = []
    for c in range(N_CORES):
        b, g = divmod(c, G)
        c0 = g * GC
        # packed [Wq | Wv'] (Wv' = Wv with a zero ones-column slot per head)
        wqvp = np.zeros((D, GC + HPG * 65), np.float32)
        wqvp[:, :GC] = Wq[:, c0:c0 + GC]
        for h in range(HPG):
                wqvp[:, GC + h * 65:GC + h * 65 + 64] = \
                    Wv[:, c0 + h * DH:c0 + (h + 1) * DH]
        bq2 = bq[c0:c0 + GC].reshape(2, 128).T
        bk2 = bk[c0:c0 + GC].reshape(2, 128).T
        b2 = np.ascontiguousarray(np.hstack([bq2, bk2]).astype(np.float32))
        in_maps.append({
                "xqT": xT[b],
                "xkvT": kvT[b],
                "wk": np.ascontiguousarray(Wk[:, c0:c0 + GC]).astype(BF16NP),
                "wqvp": wqvp.astype(BF16NP),
                "wo": np.ascontiguousarray(Wo[c0:c0 + GC, :]).astype(BF16NP),
                "b2": b2,
                "ones64": ones64,
        })
    return in_maps


def kernel(query_input, kv_input, Wq, bq, Wkv, bkv, Wo, bo):
    global _nc_cache
    from concourse import bass_utils

    if _nc_cache is None:
        _nc_cache = _build_nc()
    nc = _nc_cache

    Wkv = np.asarray(Wkv, dtype=np.float32)
    Wo = np.asarray(Wo, dtype=np.float32)
    bo = np.asarray(bo, dtype=np.float32)
    bv = np.asarray(bkv, np.float32)[D:]

    in_maps = build_in_maps(dict(
        query_input=query_input, kv_input=kv_input, Wq=Wq, bq=bq,
        Wkv=Wkv, bkv=bkv, Wo=Wo))

    res = bass_utils.run_bass_kernel_spmd(nc, in_maps,
                                          core_ids=list(range(N_CORES)))

    # gather: sum the 4 head-group partials per batch; add biases the device
    # left out (bo, and bv which passes through Wo since softmax rows sum to 1)
    tail = bv @ Wo + bo
    out = np.empty((B, SQ, D), np.float32)
    for b in range(B):
        acc = res.results[b * G + 0]["out_p"].astype(np.float32).copy()
        for g in range(1, G):
                acc += res.results[b * G + g]["out_p"]
        out[b] = acc + tail[None, :]
    return out
